# revision 2
# baseline (speedup 1.0000x reference)
"""Trainium2 Bass kernel v3 for MillionBucketPyramid hashed-embedding lookup.

v3 changes vs v2 baseline:
  - CH=8 fine-grained chunks with interleaved wave issue: c0s/c1 waves for
    chunk k+2 are issued between the t2/t3 waves of chunk k, so the Pool
    SWDGE queue (the latency-bound resource: ~1.3us per 128-descriptor
    wave, ~10ns/desc SDMA random-read floor) never idles waiting for the
    DVE logits->rehash tail.
  - DVE tail per chunk unchanged math, just narrower slices.
"""

import numpy as np

HASH_PRIMES = [2654435761, 2246822519, 3266489917, 2028178513, 1220703125, 1610612741,
               805306457, 402653189, 3674653429, 2860486313, 1073676287, 2971215073,
               1500450271, 3267000013, 2654435789, 4049292737]
M = 4_000_000
B, T, E = 32, 2048, 4
NC = 8
RPC = B // NC          # rows per core = 4
NTOK = RPC * T         # tokens per core = 8192
P = 128                # partitions
F = NTOK // P          # tokens per partition = 64
HW = 8                 # halo width
FH = F + HW            # 72
CH = 8                 # pipeline chunks
Fc = F // CH           # w-slots per chunk
PRE = 2                # chunks of short-scale waves issued ahead

_CACHE = {}


def _host_prep(tokens, table0, table1, table2, table3, cond_w):
    tokens = np.asarray(tokens)
    tok32 = tokens.astype(np.int32)
    tok_pad = np.zeros((B, T + HW), np.int32)
    tok_pad[:, HW:] = tok32

    w = np.asarray(cond_w, np.float32)

    # C0small: scale-0 embedding + its logit contribution, indexed by raw token
    v = np.arange(1024, dtype=np.int64)
    key0 = ((v * HASH_PRIMES[0]) % M).astype(np.int64)
    emb0 = np.asarray(table0, np.float32)[key0]          # [1024, 4]
    G0 = emb0 @ w[:, 0:4].T                              # [1024, 8]
    c0s = np.ascontiguousarray(np.concatenate([emb0, G0], axis=1))  # [1024, 12]

    t1 = np.asarray(table1, np.float32)
    C1 = np.concatenate([t1, t1 @ w[:, 4:8].T], axis=1)  # [4M, 12]

    # per-F-block multiplier constants for the product decomposition
    pr = np.array(HASH_PRIMES[:HW], np.int64)
    a = (pr >> 20).astype(np.int32)
    b = ((pr >> 10) & 1023).astype(np.int32)
    c = (pr & 1023).astype(np.int32)
    row = np.concatenate([np.repeat(a, F), np.repeat(b, F), np.repeat(c, F)])
    consts = np.tile(row[None, :], (P, 1)).astype(np.int32)  # [128, 1536]

    return (tok_pad, c0s, C1, np.asarray(table2, np.float32),
            np.asarray(table3, np.float32), consts)


def make_in_maps(inputs):
    tok_pad, c0s, C1, T2, T3, consts = _host_prep(**inputs)
    return [{"tok": tok_pad[c * RPC:(c + 1) * RPC],
             "c0s": c0s, "c1": C1, "t2": T2, "t3": T3, "consts": consts}
            for c in range(NC)]


def _fix_multi_waits(nc, mybir, bass_rust):
    """This walrus build allows only ONE sync-wait per instruction; split
    extras onto injected nops on the same (in-order) engine."""
    n = 0
    for f_ in nc.m.functions:
        for blk in f_.blocks:
            new_list, changed = [], False
            for inst in blk.instructions:
                si = inst.sync_info
                if si is not None and len(si.on_wait) > 1:
                    waits = list(si.on_wait)
                    for w in waits[:-1]:
                        nop = mybir.InstNoOp(name=f"waitsplit_{n}", ins=[], outs=[])
                        n += 1
                        nop.engine = inst.engine
                        nop.sync_info = bass_rust.SyncInfo(on_wait=[w], on_update=[])
                        new_list.append(nop)
                    inst.sync_info = bass_rust.SyncInfo(on_wait=[waits[-1]],
                                                        on_update=list(si.on_update))
                    changed = True
                new_list.append(inst)
            if changed:
                blk.instructions = new_list
    return n


def build_nc(fix_waits=True):
    import concourse.bass as bass
    import concourse.mybir as mybir
    import bass_rust
    from concourse.tile import TileContext

    A = mybir.AluOpType
    nc = bass.Bass()

    tok = nc.dram_tensor("tok", [RPC, T + HW], mybir.dt.int32, kind="ExternalInput")
    c0s = nc.dram_tensor("c0s", [1024, 12], mybir.dt.float32, kind="ExternalInput")
    c1 = nc.dram_tensor("c1", [M, 12], mybir.dt.float32, kind="ExternalInput")
    t2 = nc.dram_tensor("t2", [M, E], mybir.dt.float32, kind="ExternalInput")
    t3 = nc.dram_tensor("t3", [M, E], mybir.dt.float32, kind="ExternalInput")
    consts = nc.dram_tensor("consts", [P, 3 * HW * F], mybir.dt.int32,
                            kind="ExternalInput")
    out = nc.dram_tensor("out", [RPC, T, 16], mybir.dt.float32, kind="ExternalOutput")

    u32 = mybir.dt.uint32
    def u(ap):
        return ap.bitcast(u32)

    def stt_int(eng, out_, in0, scalar, op0, in1, op1):
        inst = eng.scalar_tensor_tensor(out=out_, in0=in0, scalar=scalar,
                                        op0=op0, in1=in1, op1=op1)
        inst.ins.ins[1].dtype = mybir.dt.int32
        return inst

    def igather(out_ap, table_ap, idx_ap):
        return nc.gpsimd.indirect_dma_start(
            out=out_ap, out_offset=None, in_=table_ap,
            in_offset=bass.IndirectOffsetOnAxis(ap=idx_ap, axis=0))

    with TileContext(nc) as tc:
        with tc.tile_pool(name="sbuf", bufs=1) as pool:
            i32, f32d = mybir.dt.int32, mybir.dt.float32
            W8 = HW * F                      # 512
            tokh = pool.tile([P, FH], i32, name="tokh")
            cst = pool.tile([P, 3 * W8], i32, name="cst")
            tok8 = pool.tile([P, W8], i32, name="tok8")
            X8 = pool.tile([P, W8], i32, name="X8")
            Bt = pool.tile([P, W8], i32, name="Bt")
            Ct = pool.tile([P, W8], i32, name="Ct")
            W2 = pool.tile([P, W8], i32, name="W2")
            Z8 = pool.tile([P, W8], i32, name="Z8")
            L1 = pool.tile([P, W8], i32, name="L1")
            LO8 = pool.tile([P, W8], i32, name="LO8")
            HI8 = pool.tile([P, W8], i32, name="HI8")
            LO = pool.tile([P, 3 * F], i32, name="LO")
            HI = pool.tile([P, 3 * F], i32, name="HI")
            keys = pool.tile([P, 3 * F], i32, name="keys")
            tA = pool.tile([P, 3 * F], i32, name="tA")
            tB = pool.tile([P, 3 * F], i32, name="tB")
            tC = pool.tile([P, 3 * F], i32, name="tC")
            tD = pool.tile([P, 3 * F], i32, name="tD")
            tE2 = pool.tile([P, 3 * F], i32, name="tE2")
            tF2 = pool.tile([P, 3 * F], i32, name="tF2")
            cg0 = pool.tile([P, F, 12], f32d, name="cg0")
            cg1 = pool.tile([P, F, 12], f32d, name="cg1")
            logits = pool.tile([P, F, 8], f32d, name="logits")
            masks = pool.tile([P, F, 8], i32, name="masks")
            ck = pool.tile([P, F], i32, name="ck")
            XL = pool.tile([P, 2 * F], i32, name="XL")
            fat = pool.tile([P, F, 16], f32d, name="fat")

            ts = nc.vector.tensor_scalar
            tt = nc.vector.tensor_tensor

            # 1. halo token load: partition p = r*32 + c reads tok[r, c*64 : c*64+72]
            tok_ap = bass.AP(tok if not hasattr(tok, "tensor") else tok.tensor, 0,
                             [[T + HW, RPC], [F, 32], [1, FH]])
            nc.sync.dma_start(out=tokh[:], in_=tok_ap)
            nc.sync.dma_start(out=cst[:], in_=consts[:])

            # 2. shifted token windows: block i = tokh[:, 7-i : 7-i+F]
            for i in range(HW):
                nc.vector.tensor_copy(out=tok8[:, i * F:(i + 1) * F],
                                      in_=tokh[:, HW - 1 - i:HW - 1 - i + F])

            # 3. 42-bit products v*p as (lo32 bit pattern, hi10), all int ops
            tt(out=X8[:], in0=tok8[:], in1=cst[:, 0:W8], op=A.mult)
            tt(out=Bt[:], in0=tok8[:], in1=cst[:, W8:2 * W8], op=A.mult)
            tt(out=Ct[:], in0=tok8[:], in1=cst[:, 2 * W8:3 * W8], op=A.mult)
            ts(out=W2[:], in0=Ct[:], scalar1=10, scalar2=None,
               op0=A.logical_shift_right)
            tt(out=W2[:], in0=W2[:], in1=Bt[:], op=A.add)
            ts(out=Z8[:], in0=W2[:], scalar1=10, scalar2=None,
               op0=A.logical_shift_right)
            tt(out=Z8[:], in0=Z8[:], in1=X8[:], op=A.add)
            ts(out=L1[:], in0=W2[:], scalar1=1023, scalar2=None, op0=A.bitwise_and)
            ts(out=u(L1[:]), in0=u(L1[:]), scalar1=10, scalar2=None,
               op0=A.logical_shift_left)
            stt_int(nc.vector, L1[:], Ct[:], 1023, A.bitwise_and, L1[:], A.bitwise_or)
            ts(out=X8[:], in0=Z8[:], scalar1=0xFFF, scalar2=None, op0=A.bitwise_and)
            ts(out=u(X8[:]), in0=u(X8[:]), scalar1=20, scalar2=None,
               op0=A.logical_shift_left)
            tt(out=LO8[:], in0=X8[:], in1=L1[:], op=A.bitwise_or)
            ts(out=HI8[:], in0=Z8[:], scalar1=12, scalar2=None,
               op0=A.logical_shift_right)

            # 4. XOR prefix into LO/HI; scale s (1,2,3) at cols [(s-1)*F, s*F)
            for acc, src in ((LO, LO8), (HI, HI8)):
                s1 = acc[:, 0:F]
                tt(out=s1, in0=src[:, 0:F], in1=src[:, F:2 * F], op=A.bitwise_xor)
                s2 = acc[:, F:2 * F]
                tt(out=s2, in0=s1, in1=src[:, 2 * F:3 * F], op=A.bitwise_xor)
                tt(out=s2, in0=s2, in1=src[:, 3 * F:4 * F], op=A.bitwise_xor)
                s3 = acc[:, 2 * F:3 * F]
                tt(out=s3, in0=s2, in1=src[:, 4 * F:5 * F], op=A.bitwise_xor)
                for i in (5, 6, 7):
                    tt(out=s3, in0=s3, in1=src[:, i * F:(i + 1) * F], op=A.bitwise_xor)

            # 5. mod42 cascade (batched over scales 1..3): keys = (HI*2^32+LO) mod M
            ts(out=tD[:], in0=HI[:], scalar1=11591, scalar2=None, op0=A.mult)
            ts(out=u(tD[:]), in0=u(tD[:]), scalar1=8, scalar2=None, op0=A.logical_shift_left)

            def fold32(X, Xa, Xb, Xc):
                ts(out=Xa[:], in0=X[:], scalar1=0x3FFFFF, scalar2=None, op0=A.bitwise_and)
                ts(out=u(Xc[:]), in0=u(X[:]), scalar1=22, scalar2=None, op0=A.logical_shift_right)
                ts(out=Xc[:], in0=Xc[:], scalar1=759, scalar2=None, op0=A.mult)
                ts(out=Xb[:], in0=Xc[:], scalar1=8, scalar2=None, op0=A.logical_shift_left)
                ts(out=Xc[:], in0=Xb[:], scalar1=22, scalar2=None, op0=A.logical_shift_right)
                ts(out=Xc[:], in0=Xc[:], scalar1=194304, scalar2=None, op0=A.mult)
                ts(out=Xb[:], in0=Xb[:], scalar1=0x3FFFFF, scalar2=None, op0=A.bitwise_and)

            fold32(LO, tA, tB, tC)        # aL, bL, cL
            fold32(tD, tE2, tF2, tD)      # aH, bH, cH (cH overwrites tD)
            tt(out=tA[:], in0=tA[:], in1=tB[:], op=A.add)      # s1 <= 8388606
            tt(out=tE2[:], in0=tE2[:], in1=tF2[:], op=A.add)   # s2
            tt(out=tC[:], in0=tC[:], in1=tD[:], op=A.add)      # s3 <= 15.6M
            tt(out=tA[:], in0=tA[:], in1=tE2[:], op=A.add)     # s12 <= 16777212

            def reduce_qm(S, Q):
                ts(out=Q[:], in0=S[:], scalar1=1.0 / M, scalar2=None, op0=A.mult)
                nc.vector.scalar_tensor_tensor(out=S[:], in0=Q[:], scalar=float(-M),
                                               op0=A.mult, in1=S[:], op1=A.add)

            reduce_qm(tA, tD)   # s12' in (-M, 2M)
            reduce_qm(tC, tD)   # s3'  in (-M, 2M)
            tt(out=tA[:], in0=tA[:], in1=tC[:], op=A.add)      # s in (-2M, 4M)
            reduce_qm(tA, tD)   # (-M, 2M)
            ts(out=tD[:], in0=tA[:], scalar1=0.0, scalar2=float(M), op0=A.is_lt, op1=A.mult)
            tt(out=tA[:], in0=tA[:], in1=tD[:], op=A.add)      # [0, 2M)
            ts(out=tD[:], in0=tA[:], scalar1=float(M), scalar2=float(-M), op0=A.is_ge, op1=A.mult)
            tt(out=keys[:], in0=tA[:], in1=tD[:], op=A.add)    # [0, M)

            # 6-9. chunked pipeline. Pool-queue issue order interleaves the
            # short-scale waves of chunk k+PRE between t2/t3 waves of chunk k
            # so the SWDGE queue never starves while DVE computes a tail.
            p0 = HASH_PRIMES[0] - 2**32       # two's-complement int32 immediates

            def short_waves(chk):
                lo = chk * Fc
                for w in range(lo, lo + Fc):
                    igather(cg0[:, w, :], c0s[:], tokh[:, HW - 1 + w:HW + w])
                for w in range(lo, lo + Fc):
                    igather(cg1[:, w, :], c1[:], keys[:, w:w + 1])

            def dve_tail(ch):
                sl = slice(ch * Fc, (ch + 1) * Fc)
                tt(out=logits[:, sl, :], in0=cg0[:, sl, 4:12],
                   in1=cg1[:, sl, 4:12], op=A.add)
                ts(out=masks[:, sl, :], in0=logits[:, sl, :], scalar1=0.0,
                   scalar2=-1.0, op0=A.is_gt, op1=A.mult)  # -1 where logit > 0
                ts(out=ck[:, sl], in0=masks[:, sl, 0], scalar1=p0, scalar2=None,
                   op0=A.bitwise_and)
                for i in range(1, 8):
                    pi = (HASH_PRIMES[i] if HASH_PRIMES[i] < 2**31
                          else HASH_PRIMES[i] - 2**32)
                    stt_int(nc.vector, ck[:, sl], masks[:, sl, i], pi,
                            A.bitwise_and, ck[:, sl], A.bitwise_xor)

                # rehash this chunk: XLc = [(key2^ck) mod M | (key3^ck) mod M]
                XLc = XL[:, ch * 2 * Fc:(ch + 1) * 2 * Fc]
                tt(out=XLc[:, 0:Fc], in0=keys[:, F + ch * Fc:F + (ch + 1) * Fc],
                   in1=ck[:, sl], op=A.bitwise_xor)
                tt(out=XLc[:, Fc:2 * Fc], in0=keys[:, 2 * F + ch * Fc:2 * F + (ch + 1) * Fc],
                   in1=ck[:, sl], op=A.bitwise_xor)
                co = ch * 2 * Fc
                XA = tA[:, co:co + 2 * Fc]
                XB = tB[:, co:co + 2 * Fc]
                XC = tC[:, co:co + 2 * Fc]
                XD = tD[:, co:co + 2 * Fc]
                ts(out=XA, in0=XLc[:], scalar1=0x3FFFFF, scalar2=None, op0=A.bitwise_and)
                ts(out=u(XC), in0=u(XLc[:]), scalar1=22, scalar2=None, op0=A.logical_shift_right)
                ts(out=XC, in0=XC, scalar1=759, scalar2=None, op0=A.mult)
                ts(out=XB, in0=XC, scalar1=8, scalar2=None, op0=A.logical_shift_left)
                ts(out=XC, in0=XB, scalar1=22, scalar2=None, op0=A.logical_shift_right)
                ts(out=XC, in0=XC, scalar1=194304, scalar2=None, op0=A.mult)
                ts(out=XB, in0=XB, scalar1=0x3FFFFF, scalar2=None, op0=A.bitwise_and)
                tt(out=XA, in0=XA, in1=XB, op=A.add)              # s1 <= 8.39M
                ts(out=XD, in0=XC, scalar1=float(M), scalar2=float(-M), op0=A.is_ge, op1=A.mult)
                tt(out=XC, in0=XC, in1=XD, op=A.add)              # c' <= 5.13M
                tt(out=XA, in0=XA, in1=XC, op=A.add)              # s <= 13.6M
                ts(out=XD, in0=XA, scalar1=1.0 / M, scalar2=None, op0=A.mult)
                nc.vector.scalar_tensor_tensor(out=XA, in0=XD, scalar=float(-M),
                                               op0=A.mult, in1=XA, op1=A.add)  # (-M, 2M)
                ts(out=XD, in0=XA, scalar1=0.0, scalar2=float(M), op0=A.is_lt, op1=A.mult)
                tt(out=XA, in0=XA, in1=XD, op=A.add)
                ts(out=XD, in0=XA, scalar1=float(M), scalar2=float(-M), op0=A.is_ge, op1=A.mult)
                tt(out=XLc[:], in0=XA, in1=XD, op=A.add)          # long keys [0, M)

            def long_waves(ch):
                for w in range(Fc):
                    igather(fat[:, ch * Fc + w, 8:12], t2[:],
                            XL[:, ch * 2 * Fc + w:ch * 2 * Fc + w + 1])
                for w in range(Fc):
                    igather(fat[:, ch * Fc + w, 12:16], t3[:],
                            XL[:, ch * 2 * Fc + Fc + w:ch * 2 * Fc + Fc + w + 1])

            for chk in range(min(PRE, CH)):
                short_waves(chk)
            for ch in range(CH):
                dve_tail(ch)
                if ch + PRE < CH:
                    short_waves(ch + PRE)
                long_waves(ch)

            # 10. short embeddings into the output tile
            nc.vector.tensor_copy(out=fat[:, :, 0:4], in_=cg0[:, :, 0:4])
            nc.vector.tensor_copy(out=fat[:, :, 4:8], in_=cg1[:, :, 0:4])

            # 11. store: partition p = r*32+c holds out[r, c*64:(c+1)*64, :]
            out_ap = bass.AP(out if not hasattr(out, "tensor") else out.tensor, 0,
                             [[T * 16, RPC], [F * 16, 32], [1, F * 16]])
            nc.sync.dma_start(out=out_ap, in_=fat[:])

    if fix_waits:
        _fix_multi_waits(nc, mybir, bass_rust)
    return nc


def kernel(tokens, table0, table1, table2, table3, cond_w):
    if "nc" not in _CACHE:
        _CACHE["nc"] = build_nc()
    nc = _CACHE["nc"]

    in_maps = make_in_maps(dict(tokens=tokens, table0=table0, table1=table1,
                                table2=table2, table3=table3, cond_w=cond_w))
    from concourse.bass_utils import run_bass_kernel_spmd
    res = run_bass_kernel_spmd(nc, in_maps, core_ids=list(range(NC)))
    outs = [res.results[c]["out"].reshape(RPC, T, 16) for c in range(NC)]
    return np.concatenate(outs, axis=0).astype(np.float32)


if __name__ == "__main__":
    pass


# revision 18
# speedup vs baseline: 2.0529x; 2.0529x over previous
"""Trainium2 Bass kernel v4 for MillionBucketPyramid hashed-embedding lookup.

v4 changes vs v3:
  - Scale-0 (vocab=1024) no longer uses indirect-DMA waves at all. The
    64-entry-per-partition lookup is a one-hot matmul on the otherwise-idle
    PE: OH[vocab_chunk, token] (built by DVE int-compare against an iota)
    times C0T[vocab, 24] where the 24 bf16 columns are hi/lo splits of
    G0 (logit contribution) and e0 (embedding), reconstructed in f32 by
    DVE adds. One-hot selection is exact, hi+lo recovers f32 to ~2^-17.
    This removes 64 of 256 gather waves; the SWDGE random-row drain
    (~1.3us per 128-row wave, latency-bound) is the kernel floor.
  - Wave order: c1 chunks first, then t2/t3 chunks; the PE/DVE scale-0
    pipeline and the logits->rehash tails all hide under the c1 drain.
"""

import numpy as np

HASH_PRIMES = [2654435761, 2246822519, 3266489917, 2028178513, 1220703125, 1610612741,
               805306457, 402653189, 3674653429, 2860486313, 1073676287, 2971215073,
               1500450271, 3267000013, 2654435789, 4049292737]
M = 4_000_000
B, T, E = 32, 2048, 4
NC = 8
RPC = B // NC          # rows per core = 4
NTOK = RPC * T         # tokens per core = 8192
P = 128                # partitions
F = NTOK // P          # tokens per partition = 64
HW = 8                 # halo width
FH = F + HW            # 72
CH = 8                 # pipeline chunks
Fc = F // CH           # w-slots per chunk
VC = 8                 # vocab chunks (1024 / 128)

_CACHE = {}


def _host_prep(tokens, table0, table1, table2, table3, cond_w):
    import ml_dtypes
    bf16 = ml_dtypes.bfloat16

    tokens = np.asarray(tokens)
    tok32 = tokens.astype(np.int32)
    tok_pad = np.zeros((B, T + HW), np.int32)
    tok_pad[:, HW:] = tok32

    w = np.asarray(cond_w, np.float32)

    # C0T: scale-0 matmul table [1024, 32] bf16 =
    #      [G0 hi|mid|lo (exact f32 triple split) | e0 hi|lo]
    v = np.arange(1024, dtype=np.int64)
    key0 = ((v * HASH_PRIMES[0]) % M).astype(np.int64)
    emb0 = np.asarray(table0, np.float32)[key0]          # [1024, 4]
    G0 = emb0 @ w[:, 0:4].T                              # [1024, 8] f32
    g_hi = G0.astype(bf16)
    r1 = G0 - g_hi.astype(np.float32)
    g_mid = r1.astype(bf16)
    g_lo = (r1 - g_mid.astype(np.float32)).astype(bf16)
    assert np.array_equal(
        g_hi.astype(np.float32) + g_mid.astype(np.float32)
        + g_lo.astype(np.float32),
        (g_hi.astype(np.float32) + g_mid.astype(np.float32))
        + g_lo.astype(np.float32))
    e_hi = emb0.astype(bf16)
    e_lo = (emb0 - e_hi.astype(np.float32)).astype(bf16)
    c0t = np.ascontiguousarray(
        np.concatenate([g_hi, g_mid, g_lo, e_hi, e_lo], axis=1))  # [1024, 32]

    t1 = np.asarray(table1, np.float32)
    C1 = np.concatenate([t1, t1 @ w[:, 4:8].T], axis=1)  # [4M, 12]

    # per-F-block multiplier constants for the product decomposition
    pr = np.array(HASH_PRIMES[:HW], np.int64)
    a = (pr >> 20).astype(np.int32)
    b = ((pr >> 10) & 1023).astype(np.int32)
    c = (pr & 1023).astype(np.int32)
    row = np.concatenate([np.repeat(a, F), np.repeat(b, F), np.repeat(c, F)])
    consts = np.tile(row[None, :], (P, 1)).astype(np.int32)  # [128, 1536]

    iota = np.tile(np.arange(128, dtype=np.int32).reshape(128, 1),
                   (1, NTOK // 4))

    # tokrow per core: prev-token of token (p, w) at flat position w*128 + p
    # (pure layout transform of the token input, like tok_pad)
    tokrows = []
    for c in range(NC):
        tp = tok_pad[c * RPC:(c + 1) * RPC]        # [RPC, T+HW]
        tr = np.zeros((F, P), np.int32)            # [w, p]
        for r in range(RPC):
            # prev token of (p=r*32+cc, w) = tp[r, cc*64 + w + HW-1]
            blk = tp[r, HW - 1:HW - 1 + T].reshape(32, F)   # [cc, w]
            tr[:, r * 32:(r + 1) * 32] = blk.T
        tokrows.append(np.ascontiguousarray(tr.reshape(1, NTOK)))

    return (tok_pad, c0t, C1, np.asarray(table2, np.float32),
            np.asarray(table3, np.float32), consts, iota, tokrows)


def make_in_maps(inputs):
    tok_pad, c0t, C1, T2, T3, consts, iota, tokrows = _host_prep(**inputs)
    return [{"tok": tok_pad[c * RPC:(c + 1) * RPC],
             "c0t": c0t, "c1": C1, "t2": T2, "t3": T3, "consts": consts,
             "iota": iota, "tokrow": tokrows[c]}
            for c in range(NC)]


def _fix_multi_waits(nc, mybir, bass_rust):
    """This walrus build allows only ONE sync-wait per instruction; split
    extras onto injected nops on the same (in-order) engine."""
    n = 0
    for f_ in nc.m.functions:
        for blk in f_.blocks:
            new_list, changed = [], False
            for inst in blk.instructions:
                si = inst.sync_info
                if si is not None and len(si.on_wait) > 1:
                    waits = list(si.on_wait)
                    for w in waits[:-1]:
                        nop = mybir.InstNoOp(name=f"waitsplit_{n}", ins=[], outs=[])
                        n += 1
                        nop.engine = inst.engine
                        nop.sync_info = bass_rust.SyncInfo(on_wait=[w], on_update=[])
                        new_list.append(nop)
                    inst.sync_info = bass_rust.SyncInfo(on_wait=[waits[-1]],
                                                        on_update=list(si.on_update))
                    changed = True
                new_list.append(inst)
            if changed:
                blk.instructions = new_list
    return n


def build_nc(fix_waits=True, repeat=1):
    import concourse.bass as bass
    import concourse.mybir as mybir
    import bass_rust
    from concourse.tile import TileContext

    A = mybir.AluOpType
    nc = bass.Bass()

    tok = nc.dram_tensor("tok", [RPC, T + HW], mybir.dt.int32, kind="ExternalInput")
    c0t = nc.dram_tensor("c0t", [1024, 32], mybir.dt.bfloat16, kind="ExternalInput")
    c1 = nc.dram_tensor("c1", [M, 12], mybir.dt.float32, kind="ExternalInput")
    t2 = nc.dram_tensor("t2", [M, E], mybir.dt.float32, kind="ExternalInput")
    t3 = nc.dram_tensor("t3", [M, E], mybir.dt.float32, kind="ExternalInput")
    consts = nc.dram_tensor("consts", [P, 3 * HW * F], mybir.dt.int32,
                            kind="ExternalInput")
    iota_in = nc.dram_tensor("iota", [P, NTOK // 4], mybir.dt.int32,
                             kind="ExternalInput")
    tokrow_in = nc.dram_tensor("tokrow", [1, NTOK], mybir.dt.int32,
                               kind="ExternalInput")
    out = nc.dram_tensor("out", [RPC, T, 16], mybir.dt.float32, kind="ExternalOutput")

    u32 = mybir.dt.uint32
    def u(ap):
        return ap.bitcast(u32)

    def stt_int(eng, out_, in0, scalar, op0, in1, op1):
        inst = eng.scalar_tensor_tensor(out=out_, in0=in0, scalar=scalar,
                                        op0=op0, in1=in1, op1=op1)
        inst.ins.ins[1].dtype = mybir.dt.int32
        return inst

    def igather(out_ap, table_ap, idx_ap):
        return nc.gpsimd.indirect_dma_start(
            out=out_ap, out_offset=None, in_=table_ap,
            in_offset=bass.IndirectOffsetOnAxis(ap=idx_ap, axis=0))

    with TileContext(nc) as tc:
        with tc.tile_pool(name="sbuf", bufs=1) as pool, \
             tc.tile_pool(name="psum", bufs=1, space="PSUM") as ppool:
            i32, f32d, bf = mybir.dt.int32, mybir.dt.float32, mybir.dt.bfloat16
            W8 = HW * F                      # 512
            tokh = pool.tile([P, FH], i32, name="tokh")
            cst = pool.tile([P, 3 * W8], i32, name="cst")
            iota = pool.tile([P, NTOK // 4], i32, name="iota")
            tokrep = pool.tile([P, NTOK], i32, name="tokrep")
            c0ts = pool.tile([P, VC, 32], bf, name="c0ts")
            QT = 4
            oh = [pool.tile([P, NTOK // QT], bf, name=f"oh{b}")
                  for b in range(VC)]
            psum = ppool.tile([P, F, 32], f32d, name="psum")
            tok8 = pool.tile([P, W8], i32, name="tok8")
            X8 = pool.tile([P, W8], i32, name="X8")
            Bt = pool.tile([P, W8], i32, name="Bt")
            Ct = pool.tile([P, W8], i32, name="Ct")
            W2 = pool.tile([P, W8], i32, name="W2")
            Z8 = pool.tile([P, W8], i32, name="Z8")
            L1 = pool.tile([P, W8], i32, name="L1")
            LO8 = pool.tile([P, W8], i32, name="LO8")
            HI8 = pool.tile([P, W8], i32, name="HI8")
            LO = pool.tile([P, 3 * F], i32, name="LO")
            HI = pool.tile([P, 3 * F], i32, name="HI")
            keys = pool.tile([P, 3 * F], i32, name="keys")
            tA = pool.tile([P, 3 * F], i32, name="tA")
            tB = pool.tile([P, 3 * F], i32, name="tB")
            tC = pool.tile([P, 3 * F], i32, name="tC")
            tD = pool.tile([P, 3 * F], i32, name="tD")
            tE2 = pool.tile([P, 3 * F], i32, name="tE2")
            tF2 = pool.tile([P, 3 * F], i32, name="tF2")
            cg1 = pool.tile([P, F, 12], f32d, name="cg1")
            logits = pool.tile([P, F, 8], f32d, name="logits")
            masks = pool.tile([P, F, 8], i32, name="masks")
            ck = pool.tile([P, F], i32, name="ck")
            XL = pool.tile([P, 2 * F], i32, name="XL")
            fat = pool.tile([P, F, 16], f32d, name="fat")

            ts = nc.vector.tensor_scalar
            tt = nc.vector.tensor_tensor

            for _rep in range(repeat):
                # 1. loads. tokh: partition p = r*32+c reads tok[r, c*64 : c*64+72].
                # tokrow: flat [1, 8192] of PREVIOUS tokens, order j = w*128 + p.
                tok_ap = bass.AP(tok if not hasattr(tok, "tensor") else tok.tensor, 0,
                                 [[T + HW, RPC], [F, 32], [1, FH]])
                nc.sync.dma_start(out=tokh[:], in_=tok_ap)
                nc.sync.dma_start(out=cst[:], in_=consts[:])
                nc.sync.dma_start(out=iota[:], in_=iota_in[:])
                nc.sync.dma_start(out=tokrep[:],
                                  in_=tokrow_in[:].to_broadcast((P, NTOK)))
                c0t_ap = bass.AP(c0t if not hasattr(c0t, "tensor") else c0t.tensor, 0,
                                 [[32, P], [P * 32, VC], [1, 32]])
                nc.sync.dma_start(out=c0ts[:], in_=c0t_ap)

                # 2. shifted token windows: block i = tokh[:, 7-i : 7-i+F]
                for i in range(HW):
                    nc.vector.tensor_copy(out=tok8[:, i * F:(i + 1) * F],
                                          in_=tokh[:, HW - 1 - i:HW - 1 - i + F])

                # 3. 42-bit products v*p as (lo32 bit pattern, hi10), all int ops
                tt(out=X8[:], in0=tok8[:], in1=cst[:, 0:W8], op=A.mult)
                tt(out=Bt[:], in0=tok8[:], in1=cst[:, W8:2 * W8], op=A.mult)
                tt(out=Ct[:], in0=tok8[:], in1=cst[:, 2 * W8:3 * W8], op=A.mult)
                ts(out=W2[:], in0=Ct[:], scalar1=10, scalar2=None,
                   op0=A.logical_shift_right)
                tt(out=W2[:], in0=W2[:], in1=Bt[:], op=A.add)
                ts(out=Z8[:], in0=W2[:], scalar1=10, scalar2=None,
                   op0=A.logical_shift_right)
                tt(out=Z8[:], in0=Z8[:], in1=X8[:], op=A.add)
                ts(out=L1[:], in0=W2[:], scalar1=1023, scalar2=None, op0=A.bitwise_and)
                ts(out=u(L1[:]), in0=u(L1[:]), scalar1=10, scalar2=None,
                   op0=A.logical_shift_left)
                stt_int(nc.vector, L1[:], Ct[:], 1023, A.bitwise_and, L1[:], A.bitwise_or)
                ts(out=X8[:], in0=Z8[:], scalar1=0xFFF, scalar2=None, op0=A.bitwise_and)
                ts(out=u(X8[:]), in0=u(X8[:]), scalar1=20, scalar2=None,
                   op0=A.logical_shift_left)
                tt(out=LO8[:], in0=X8[:], in1=L1[:], op=A.bitwise_or)
                ts(out=HI8[:], in0=Z8[:], scalar1=12, scalar2=None,
                   op0=A.logical_shift_right)

                # 4. XOR prefix into LO/HI; scale s (1,2,3) at cols [(s-1)*F, s*F)
                for acc, src in ((LO, LO8), (HI, HI8)):
                    s1 = acc[:, 0:F]
                    tt(out=s1, in0=src[:, 0:F], in1=src[:, F:2 * F], op=A.bitwise_xor)
                    s2 = acc[:, F:2 * F]
                    tt(out=s2, in0=s1, in1=src[:, 2 * F:3 * F], op=A.bitwise_xor)
                    tt(out=s2, in0=s2, in1=src[:, 3 * F:4 * F], op=A.bitwise_xor)
                    s3 = acc[:, 2 * F:3 * F]
                    tt(out=s3, in0=s2, in1=src[:, 4 * F:5 * F], op=A.bitwise_xor)
                    for i in (5, 6, 7):
                        tt(out=s3, in0=s3, in1=src[:, i * F:(i + 1) * F], op=A.bitwise_xor)

                # 5. mod42 cascade (batched over scales 1..3): keys = (HI*2^32+LO) mod M
                ts(out=tD[:], in0=HI[:], scalar1=11591, scalar2=None, op0=A.mult)
                ts(out=u(tD[:]), in0=u(tD[:]), scalar1=8, scalar2=None, op0=A.logical_shift_left)

                def fold32(X, Xa, Xb, Xc):
                    ts(out=Xa[:], in0=X[:], scalar1=0x3FFFFF, scalar2=None, op0=A.bitwise_and)
                    ts(out=u(Xc[:]), in0=u(X[:]), scalar1=22, scalar2=None, op0=A.logical_shift_right)
                    ts(out=Xc[:], in0=Xc[:], scalar1=759, scalar2=None, op0=A.mult)
                    ts(out=Xb[:], in0=Xc[:], scalar1=8, scalar2=None, op0=A.logical_shift_left)
                    ts(out=Xc[:], in0=Xb[:], scalar1=22, scalar2=None, op0=A.logical_shift_right)
                    ts(out=Xc[:], in0=Xc[:], scalar1=194304, scalar2=None, op0=A.mult)
                    ts(out=Xb[:], in0=Xb[:], scalar1=0x3FFFFF, scalar2=None, op0=A.bitwise_and)

                fold32(LO, tA, tB, tC)        # aL, bL, cL
                fold32(tD, tE2, tF2, tD)      # aH, bH, cH (cH overwrites tD)
                tt(out=tA[:], in0=tA[:], in1=tB[:], op=A.add)      # s1 <= 8388606
                tt(out=tE2[:], in0=tE2[:], in1=tF2[:], op=A.add)   # s2
                tt(out=tC[:], in0=tC[:], in1=tD[:], op=A.add)      # s3 <= 15.6M
                tt(out=tA[:], in0=tA[:], in1=tE2[:], op=A.add)     # s12 <= 16777212

                def reduce_qm(S, Q):
                    ts(out=Q[:], in0=S[:], scalar1=1.0 / M, scalar2=None, op0=A.mult)
                    nc.vector.scalar_tensor_tensor(out=S[:], in0=Q[:], scalar=float(-M),
                                                   op0=A.mult, in1=S[:], op1=A.add)

                reduce_qm(tA, tD)   # s12' in (-M, 2M)
                reduce_qm(tC, tD)   # s3'  in (-M, 2M)
                tt(out=tA[:], in0=tA[:], in1=tC[:], op=A.add)      # s in (-2M, 4M)
                reduce_qm(tA, tD)   # (-M, 2M)
                ts(out=tD[:], in0=tA[:], scalar1=0.0, scalar2=float(M), op0=A.is_lt, op1=A.mult)
                tt(out=tA[:], in0=tA[:], in1=tD[:], op=A.add)      # [0, 2M)
                ts(out=tD[:], in0=tA[:], scalar1=float(M), scalar2=float(-M), op0=A.is_ge, op1=A.mult)
                tt(out=keys[:], in0=tA[:], in1=tD[:], op=A.add)    # [0, M)

                # 6. scale-0 one-hot matmul: OH chunks on DVE, accumulate PSUM.
                # psum[:, w, 0:8]=G0_hi, 8:16=G0_lo, 16:20=e0_hi, 20:24=e0_lo.
                QW = NTOK // QT          # tokens per quarter
                FQ = F // QT             # w-slots per quarter
                iota_b = iota[:]
                for q in range(QT):
                    for ci in range(VC):
                        nc.vector.scalar_tensor_tensor(
                            out=oh[ci][:], in0=tokrep[:, q * QW:(q + 1) * QW],
                            scalar=float(-(ci * P)), op0=A.add,
                            in1=iota_b, op1=A.is_equal)
                    for wl in range(FQ):
                        w = q * FQ + wl
                        for ci in range(VC):
                            nc.tensor.matmul(
                                psum[:, w, 0:32],
                                oh[ci][:, wl * P:(wl + 1) * P],
                                c0ts[:, ci, :],
                                start=(ci == 0), stop=(ci == VC - 1))

                # 7. c1 gather waves — [128,1]-index, one per w
                for w in range(F):
                    igather(cg1[:, w, :], c1[:], keys[:, w:w + 1])

                # 8. per-chunk: logits -> sign bits -> cond key -> rehash
                p0 = HASH_PRIMES[0] - 2**32   # two's-complement int32 immediates

                def dve_tail(ch):
                    sl = slice(ch * Fc, (ch + 1) * Fc)
                    tt(out=logits[:, sl, :], in0=cg1[:, sl, 4:12],
                       in1=psum[:, sl, 0:8], op=A.add)
                    tt(out=logits[:, sl, :], in0=logits[:, sl, :],
                       in1=psum[:, sl, 8:16], op=A.add)
                    tt(out=logits[:, sl, :], in0=logits[:, sl, :],
                       in1=psum[:, sl, 16:24], op=A.add)
                    nc.vector.tensor_copy(out=fat[:, sl, 0:4],
                                          in_=psum[:, sl, 24:28])
                    tt(out=fat[:, sl, 0:4], in0=fat[:, sl, 0:4],
                       in1=psum[:, sl, 28:32], op=A.add)
                    ts(out=masks[:, sl, :], in0=logits[:, sl, :], scalar1=0.0,
                       scalar2=-1.0, op0=A.is_gt, op1=A.mult)  # -1 where logit > 0
                    ts(out=ck[:, sl], in0=masks[:, sl, 0], scalar1=p0, scalar2=None,
                       op0=A.bitwise_and)
                    for i in range(1, 8):
                        pi = (HASH_PRIMES[i] if HASH_PRIMES[i] < 2**31
                              else HASH_PRIMES[i] - 2**32)
                        stt_int(nc.vector, ck[:, sl], masks[:, sl, i], pi,
                                A.bitwise_and, ck[:, sl], A.bitwise_xor)

                    # rehash: XLc = [(key2^ck) mod M | (key3^ck) mod M]
                    XLc = XL[:, ch * 2 * Fc:(ch + 1) * 2 * Fc]
                    tt(out=XLc[:, 0:Fc], in0=keys[:, F + ch * Fc:F + (ch + 1) * Fc],
                       in1=ck[:, sl], op=A.bitwise_xor)
                    tt(out=XLc[:, Fc:2 * Fc], in0=keys[:, 2 * F + ch * Fc:2 * F + (ch + 1) * Fc],
                       in1=ck[:, sl], op=A.bitwise_xor)
                    co = ch * 2 * Fc
                    XA = tA[:, co:co + 2 * Fc]
                    XB = tB[:, co:co + 2 * Fc]
                    XC = tC[:, co:co + 2 * Fc]
                    XD = tD[:, co:co + 2 * Fc]
                    ts(out=XA, in0=XLc[:], scalar1=0x3FFFFF, scalar2=None, op0=A.bitwise_and)
                    ts(out=u(XC), in0=u(XLc[:]), scalar1=22, scalar2=None, op0=A.logical_shift_right)
                    ts(out=XC, in0=XC, scalar1=759, scalar2=None, op0=A.mult)
                    ts(out=XB, in0=XC, scalar1=8, scalar2=None, op0=A.logical_shift_left)
                    ts(out=XC, in0=XB, scalar1=22, scalar2=None, op0=A.logical_shift_right)
                    ts(out=XC, in0=XC, scalar1=194304, scalar2=None, op0=A.mult)
                    ts(out=XB, in0=XB, scalar1=0x3FFFFF, scalar2=None, op0=A.bitwise_and)
                    tt(out=XA, in0=XA, in1=XB, op=A.add)              # s1 <= 8.39M
                    ts(out=XD, in0=XC, scalar1=float(M), scalar2=float(-M), op0=A.is_ge, op1=A.mult)
                    tt(out=XC, in0=XC, in1=XD, op=A.add)              # c' <= 5.13M
                    tt(out=XA, in0=XA, in1=XC, op=A.add)              # s <= 13.6M
                    ts(out=XD, in0=XA, scalar1=1.0 / M, scalar2=None, op0=A.mult)
                    nc.vector.scalar_tensor_tensor(out=XA, in0=XD, scalar=float(-M),
                                                   op0=A.mult, in1=XA, op1=A.add)  # (-M, 2M)
                    ts(out=XD, in0=XA, scalar1=0.0, scalar2=float(M), op0=A.is_lt, op1=A.mult)
                    tt(out=XA, in0=XA, in1=XD, op=A.add)
                    ts(out=XD, in0=XA, scalar1=float(M), scalar2=float(-M), op0=A.is_ge, op1=A.mult)
                    tt(out=XLc[:], in0=XA, in1=XD, op=A.add)          # long keys [0, M)

                def long_waves(ch):
                    for w in range(Fc):
                        igather(fat[:, ch * Fc + w, 8:12], t2[:],
                                XL[:, ch * 2 * Fc + w:ch * 2 * Fc + w + 1])
                    for w in range(Fc):
                        igather(fat[:, ch * Fc + w, 12:16], t3[:],
                                XL[:, ch * 2 * Fc + Fc + w:ch * 2 * Fc + Fc + w + 1])

                for ch in range(CH):
                    dve_tail(ch)
                    long_waves(ch)

                # 10. scale-1 embedding into the output tile
                nc.vector.tensor_copy(out=fat[:, :, 4:8], in_=cg1[:, :, 0:4])

                # 11. store: partition p = r*32+c holds out[r, c*64:(c+1)*64, :]
                out_ap = bass.AP(out if not hasattr(out, "tensor") else out.tensor, 0,
                                 [[T * 16, RPC], [F * 16, 32], [1, F * 16]])
                nc.sync.dma_start(out=out_ap, in_=fat[:])

    if fix_waits:
        _fix_multi_waits(nc, mybir, bass_rust)
    return nc


def kernel(tokens, table0, table1, table2, table3, cond_w):
    if "nc" not in _CACHE:
        _CACHE["nc"] = build_nc()
    nc = _CACHE["nc"]

    in_maps = make_in_maps(dict(tokens=tokens, table0=table0, table1=table1,
                                table2=table2, table3=table3, cond_w=cond_w))
    from concourse.bass_utils import run_bass_kernel_spmd
    res = run_bass_kernel_spmd(nc, in_maps, core_ids=list(range(NC)))
    outs = [res.results[c]["out"].reshape(RPC, T, 16) for c in range(NC)]
    return np.concatenate(outs, axis=0).astype(np.float32)


if __name__ == "__main__":
    pass
